# revision 9
# baseline (speedup 1.0000x reference)
"""Bottleneck_DCN_MPCA Trainium2 Bass kernel (8-core SPMD).

Sharding: 8 shards = (batch b in {0,1}) x (H-quarter q in {0..3}); each core
computes 20 output rows of one batch. Host preprocesses full inputs into
per-core tensors and reassembles per-core outputs.

Per dcn_mpca layer, on device:
  - offset conv as 9-tap shifted-AP matmuls (bf16, f32 PSUM accumulation)
  - pooled stats via a per-batch-group AllGather + on-device combine
  - MPCA gate convs (tiny, f32) replicated per core; gating of the local shard
  - per-tap floor/frac/validity/coefficient/index math in a packed layout
    [117 partitions = 9 taps x 13 pixel-groups, (y|x|m) column blocks]
  - bilinear gather via dma_gather (transpose=True): elem = 2 horizontally
    adjacent pixels x C channels from a channels-last padded HBM buffer
  - per-pixel coefficients (bilinear x mask x validity) broadcast across
    partitions via K=9 PE matmuls; applied with one tensor_tensor multiply
  - 4 bilinear corners folded into the PE contraction (PSUM accumulation)
  - BN+SiLU epilogue on ACT
Between layers: per-batch-group AllGather of the layer-1 output
(channels-last main block + channels-first halo rows in one buffer).
"""
import os
import numpy as np
from ml_dtypes import bfloat16

import concourse.bass as bass
import concourse.bacc as bacc
import concourse.mybir as mybir
from concourse.tile import TileContext
from concourse.bass_utils import run_bass_kernel_spmd

H = W = 80
ROWS = 20
PIX = ROWS * W            # 1600
PIXP = 1664               # 13*128
KK = 9
EPS = 1e-5
NPAD = (H + 2) * W + 2    # 6562
GK = 63.5                 # floor(p) = round(p + 63.5) - 64
PASSES = [(0, 768), (768, 896)]
PASS_NCHUNKS = {768: [(0, 512), (512, 256)], 896: [(0, 512), (512, 384)]}
PASS_WOFF = {0: 0, 768: 48}
OMCH = [(0, 6), (480, 6), (960, 6), (1440, 2)]
SQ = (ROWS + 2) * (W + 2)  # 1804
BLK = (PIXP + 160) * 128   # x2 collective per-rank elements

f32 = mybir.dt.float32
bf16 = mybir.dt.bfloat16
i16 = mybir.dt.int16
A = mybir.AluOpType
AF = mybir.ActivationFunctionType
AX = mybir.AxisListType

DEBUG = bool(int(os.environ.get("DCN_DEBUG", "0")))
_CACHE = {}


def rap(ap, dims, extra_off=0):
    """Raw AP over the backing tensor of a tile/tensor AP."""
    return bass.AP(ap.tensor, ap.offset + extra_off, dims)


# ================================================================ host prep ==

def _sigmoid(x):
    return 1.0 / (1.0 + np.exp(-x))


def _perm():
    return np.array([2 * k for k in range(9)] + [2 * k + 1 for k in range(9)]
                    + [18 + k for k in range(9)], np.int64)


def _fold_bn(g, b, m, v):
    s = np.asarray(g, np.float32) / np.sqrt(np.asarray(v, np.float32) + EPS)
    return s.astype(np.float32), \
        (np.asarray(b, np.float32) - np.asarray(m, np.float32) * s).astype(np.float32)


def _prep_layer(p):
    perm = _perm()
    out = {}
    ow = np.asarray(p["off_w"], np.float32)[perm]
    Cin = ow.shape[1]
    CH = Cin // 128
    offl = np.zeros((128, KK * CH * 27), np.float32)
    for k in range(KK):
        for ch in range(CH):
            t = k * CH + ch
            offl[:, t * 27:(t + 1) * 27] = ow[:, ch * 128:(ch + 1) * 128,
                                              k // 3, k % 3].T
    out["offw"] = offl.astype(bfloat16)
    out["ob"] = np.asarray(p["off_b"], np.float32)[perm].reshape(27, 1)

    for name, key in (("gap", "gap"), ("hw", "hw"), ("pool_hw", "pool")):
        cb = p["mpca"][name]
        wgt = np.asarray(cb["w"], np.float32)[perm][:, perm]
        s, b = _fold_bn(cb["bn"]["g"], cb["bn"]["b"], cb["bn"]["m"], cb["bn"]["v"])
        if name == "hw":
            hw = np.zeros((27, 81), np.float32)
            for t in range(3):
                hw[:, t * 27:(t + 1) * 27] = wgt[:, :, t, 0].T
            out["hww"] = hw
        else:
            out[f"{key}w"] = np.ascontiguousarray(wgt[:, :, 0, 0].T)
        out[f"{key}s"] = s.reshape(27, 1)
        out[f"{key}b"] = b.reshape(27, 1)

    wd = np.asarray(p["w"], np.float32)
    Cout = wd.shape[0]
    MB = Cout // 128
    wk = wd.reshape(Cout, Cin, KK)
    wl = np.zeros((128, 18 * 128), np.float32)
    for k in range(KK):
        if MB == 1:
            for ch in range(CH):
                t = k * CH + ch
                wl[:, t * 128:(t + 1) * 128] = wk[:, ch * 128:(ch + 1) * 128, k].T
        else:
            for mb in range(MB):
                t = k * MB + mb
                wl[:, t * 128:(t + 1) * 128] = wk[mb * 128:(mb + 1) * 128, :, k].T
    out["ww"] = wl.astype(bfloat16)
    s, b = _fold_bn(p["bn"]["g"], p["bn"]["b"], p["bn"]["m"], p["bn"]["v"])
    b = b + s * np.asarray(p["b"], np.float32)
    out["bns"] = s.reshape(Cout, 1).astype(np.float32)
    out["bnb"] = b.reshape(Cout, 1).astype(np.float32)
    out["Cin"], out["Cout"], out["CH"], out["MB"] = Cin, Cout, CH, MB
    return out


def _make_xcl(x_b):
    C = x_b.shape[0]
    buf = np.zeros((NPAD, C), np.float32)
    buf[W + 1:W + 1 + H * W] = x_b.reshape(C, H * W).T
    return buf.astype(bfloat16)


def _make_xsq(x_b, r0):
    C = x_b.shape[0]
    CH = C // 128
    xt = np.zeros((C, ROWS + 2, W + 2), np.float32)
    lo, hi = r0 - 1, r0 + ROWS + 1
    slo, shi = max(lo, 0), min(hi, H)
    xt[:, slo - lo:shi - lo, 1:W + 1] = x_b[:, slo:shi, :]
    xt = xt.reshape(CH, 128, SQ)
    return np.ascontiguousarray(
        np.transpose(xt, (1, 0, 2)).reshape(128, CH * SQ)).astype(bfloat16)


def _make_base_pk(r0):
    pixr = np.arange(PIXP)
    hh = r0 + np.minimum(pixr, PIX - 1) // W
    ww = np.minimum(pixr, PIX - 1) % W
    ky = np.repeat(np.arange(3), 3)
    kx = np.tile(np.arange(3), 3)
    by = (ky[:, None] - 1 + hh[None, :]).astype(np.float32) + GK
    bx = (kx[:, None] - 1 + ww[None, :]).astype(np.float32) + GK
    out = np.zeros((117, 256), np.float32)
    for k in range(KK):
        for g in range(13):
            out[k * 13 + g, 0:128] = by[k, g * 128:(g + 1) * 128]
            out[k * 13 + g, 128:256] = bx[k, g * 128:(g + 1) * 128]
    return out


def _make_sel():
    sel = np.zeros((9, 9 * 128), np.float32)
    for k in range(KK):
        sel[k, k * 128:(k + 1) * 128] = 1.0
    return sel.astype(bfloat16)


# ============================================================ device program ==

def _emit_layer(nc, tc, li, cst, dram, meta, io):
    CH, MB = meta["CH"], meta["MB"]
    name = f"l{li}"

    with tc.tile_pool(name=f"{name}_om", bufs=1) as omp, \
         tc.tile_pool(name=f"{name}_ompsum", bufs=2, space="PSUM") as omps, \
         tc.tile_pool(name=f"{name}_sc", bufs=1) as scp:

        # ---------- offset conv ----------
        om = omp.tile([27, PIXP], f32)
        nc.vector.memset(om[:], 0.0)
        if io.get("xsq_src") is not None:
            xsqt = omp.tile([128, CH * SQ], bf16, tag="xsqt", name="xsqt")
            nc.sync.dma_start(out=xsqt[:], in_=io["xsq_src"][:])
            xsq = xsqt[:]
        else:
            xsq = io["xsq"]
        for (coff, nrows) in OMCH:
            pom = omps.tile([27, 512], f32, tag="pom", name="pom")
            n = nrows * W
            r0row = coff // W
            nt = KK * CH
            for t in range(nt):
                k, ch = t // CH, t % CH
                ky, kx = k // 3, k % 3
                x3 = xsq[:, ch * SQ:(ch + 1) * SQ].rearrange(
                    "p (r c) -> p r c", c=82)
                rhs = x3[:, ky + r0row:ky + r0row + nrows, kx:kx + W]
                nc.tensor.matmul(pom[:, :n],
                                 cst[f"offw{li}"][:, t * 27:(t + 1) * 27],
                                 rhs, start=(t == 0), stop=(t == nt - 1))
            nc.scalar.activation(om[:, coff:coff + n], pom[:, :n],
                                 AF.Identity, bias=cst[f"ob{li}"][:], scale=1.0)

        # ---------- pooled stats ----------
        stin = omp.tile([27, 100], f32)
        nc.vector.tensor_reduce(stin[:, 0:20],
                                om[:, 0:PIX].rearrange("p (r c) -> p r c", c=W),
                                axis=AX.X, op=A.add)
        nc.vector.tensor_reduce(stin[:, 20:100],
                                om[:, 0:PIX].rearrange("p (r c) -> p c r", c=W),
                                axis=AX.X, op=A.add)
        ag_i, ag_o = dram[f"ag{li}_in"], dram[f"ag{li}_out"]
        nc.sync.dma_start(out=ag_i[:], in_=stin[:])
        nc.gpsimd.collective_compute(
            "AllGather", A.bypass, replica_groups=[[0, 1, 2, 3], [4, 5, 6, 7]],
            ins=[ag_i.opt()], outs=[ag_o.opt()])
        sb4 = omp.tile([27, 400], f32)
        nc.sync.dma_start(out=sb4[:],
                          in_=rap(ag_o[:], [[100, 27], [2700, 4], [1, 100]]))
        sb4v = sb4[:].rearrange("p (r c) -> p r c", c=100)

        st160 = omp.tile([27, 162], f32)
        nc.vector.memset(st160[:], 0.0)
        nc.vector.tensor_copy(
            st160[:, 1:81].rearrange("p (r c) -> p r c", c=20), sb4v[:, :, 0:20])
        pwt = omp.tile([27, 80], f32)
        nc.vector.tensor_tensor(pwt[:], sb4v[:, 0, 20:100], sb4v[:, 1, 20:100],
                                A.add)
        nc.vector.tensor_tensor(pwt[:], pwt[:], sb4v[:, 2, 20:100], A.add)
        nc.vector.tensor_tensor(st160[:, 81:161], pwt[:], sb4v[:, 3, 20:100],
                                A.add)
        nc.vector.tensor_scalar_mul(st160[:, 1:161], st160[:, 1:161], 1.0 / 80)

        # ---------- MPCA ----------
        pch = omp.tile([27, 1], f32)
        nc.vector.tensor_reduce(pch[:], st160[:, 1:81], axis=AX.X, op=A.add)
        nc.vector.tensor_scalar_mul(pch[:], pch[:], 1.0 / 80)
        pmp = omps.tile([27, 160], f32, tag="pmp", name="pmp")
        nc.tensor.matmul(pmp[:, 0:1], cst[f"gapw{li}"][:], pch[:],
                         start=True, stop=True)
        pchf = omp.tile([27, 1], f32)
        nc.scalar.activation(pchf[:], pmp[:, 0:1], AF.Silu,
                             bias=cst[f"gapb{li}"][:], scale=cst[f"gaps{li}"][:])
        pmp2 = omps.tile([27, 160], f32, tag="pmp", name="pmp2")
        for t in range(3):
            nc.tensor.matmul(pmp2[:], cst[f"hww{li}"][:, t * 27:(t + 1) * 27],
                             st160[:, t:t + 160], start=(t == 0), stop=(t == 2))
        phw = omp.tile([27, 160], f32)
        nc.scalar.activation(phw[:], pmp2[:], AF.Silu,
                             bias=cst[f"hwb{li}"][:], scale=cst[f"hws{li}"][:])
        pmp3 = omps.tile([27, 160], f32, tag="pmp", name="pmp3")
        nc.tensor.matmul(pmp3[:], cst[f"poolw{li}"][:], phw[:],
                         start=True, stop=True)
        wgt = omp.tile([27, 160], f32)
        nc.scalar.activation(wgt[:], pmp3[:], AF.Silu,
                             bias=cst[f"poolb{li}"][:], scale=cst[f"pools{li}"][:])
        nc.scalar.activation(wgt[:], wgt[:], AF.Sigmoid)
        wm = omp.tile([27, 1], f32)
        nc.vector.tensor_reduce(wm[:], wgt[:], axis=AX.X, op=A.add)
        spch = omp.tile([27, 1], f32)
        nc.vector.scalar_tensor_tensor(spch[:], wm[:], 1.0 / 160, pchf[:],
                                       op0=A.mult, op1=A.mult)
        nc.scalar.activation(spch[:], spch[:], AF.Sigmoid)
        sph = omp.tile([27, 80], f32)
        nc.vector.tensor_tensor(sph[:], phw[:, 0:80], wgt[:, 0:80], A.mult)
        nc.scalar.activation(sph[:], sph[:], AF.Sigmoid)
        spw = omp.tile([27, 80], f32)
        nc.vector.tensor_tensor(spw[:], phw[:, 80:160], wgt[:, 80:160], A.mult)
        nc.scalar.activation(spw[:], spw[:], AF.Sigmoid)
        spl4 = omp.tile([27, 80], f32)
        nc.vector.tensor_tensor(
            spl4[:].rearrange("p (r c) -> p r c", c=20),
            sph[:].rearrange("p (r c) -> p r c", c=20),
            cst["qsel"][:].unsqueeze(2).broadcast_to([27, 4, 20]), A.mult)
        sphl = omp.tile([27, 20], f32)
        nc.vector.tensor_reduce(sphl[:],
                                spl4[:].rearrange("p (r c) -> p c r", c=20),
                                axis=AX.X, op=A.add)

        # ---------- gate the local om shard ----------
        omv = om[:, 0:PIX].rearrange("p (r c) -> p r c", c=W)
        nc.vector.tensor_tensor(
            omv, omv, sphl[:].unsqueeze(2).broadcast_to([27, ROWS, W]), A.mult)
        nc.vector.tensor_tensor(
            omv, omv, spw[:].unsqueeze(1).broadcast_to([27, ROWS, W]), A.mult)
        nc.vector.tensor_scalar_mul(om[:, 0:PIX], om[:, 0:PIX], spch[:])
        if DEBUG:
            nc.sync.dma_start(out=io["dbg_om"][:], in_=om[:])

        # ---------- stage C (packed) ----------
        pk = scp.tile([117, 384], f32)
        for grp in range(3):
            nc.sync.dma_start(
                out=pk[:, grp * 128:(grp + 1) * 128],
                in_=om[grp * 9:(grp + 1) * 9, :].rearrange(
                    "p (g c) -> p g c", c=128))
        nc.scalar.activation(pk[:, 256:384], pk[:, 256:384], AF.Sigmoid)
        qt = scp.tile([117, 256], f32)
        nc.vector.tensor_tensor(qt[:], pk[:, 0:256], cst["base"][:], A.add)
        qi = scp.tile([117, 256], i16)
        nc.vector.tensor_copy(qi[:], qt[:])
        qf = scp.tile([117, 256], f32)
        nc.vector.tensor_copy(qf[:], qi[:])
        wq = scp.tile([117, 256], f32)
        nc.vector.scalar_tensor_tensor(wq[:], qf[:], -1.0, qt[:],
                                       op0=A.mult, op1=A.add)
        v0 = scp.tile([117, 256], f32)
        nc.vector.tensor_scalar(out=v0[:], in0=qf[:], scalar1=64.0,
                                scalar2=None, op0=A.is_ge)
        nc.vector.scalar_tensor_tensor(v0[:], qf[:], 143.0, v0[:],
                                       op0=A.is_le, op1=A.mult)
        v1 = scp.tile([117, 256], f32)
        nc.vector.tensor_scalar(out=v1[:], in0=qf[:], scalar1=63.0,
                                scalar2=None, op0=A.is_ge)
        nc.vector.scalar_tensor_tensor(v1[:], qf[:], 142.0, v1[:],
                                       op0=A.is_le, op1=A.mult)
        u0 = scp.tile([117, 256], f32)
        nc.vector.tensor_scalar(out=u0[:], in0=wq[:], scalar1=-1.0, scalar2=0.5,
                                op0=A.mult, op1=A.add)
        nc.vector.tensor_tensor(u0[:], u0[:], v0[:], A.mult)
        u1 = scp.tile([117, 256], f32)
        nc.vector.tensor_scalar(out=u1[:], in0=wq[:], scalar1=0.5, scalar2=None,
                                op0=A.add)
        nc.vector.tensor_tensor(u1[:], u1[:], v1[:], A.mult)
        qc = scp.tile([117, 256], f32)
        nc.vector.tensor_scalar(out=qc[:], in0=qf[:], scalar1=143.0,
                                scalar2=63.0, op0=A.min, op1=A.max)
        xm0 = scp.tile([117, 128], f32)
        nc.vector.tensor_tensor(xm0[:], u0[:, 128:256], pk[:, 256:384], A.mult)
        xm1 = scp.tile([117, 128], f32)
        nc.vector.tensor_tensor(xm1[:], u1[:, 128:256], pk[:, 256:384], A.mult)
        cc = {}
        for (yc, xc) in ((0, 0), (0, 1), (1, 0), (1, 1)):
            t = scp.tile([117, 128], bf16, tag=f"cc{yc}{xc}", name=f"cc{yc}{xc}")
            uy = u0 if yc == 0 else u1
            xm = xm0 if xc == 0 else xm1
            nc.vector.tensor_tensor(t[:], uy[:, 0:128], xm[:], A.mult)
            cc[(yc, xc)] = t
        s80 = scp.tile([117, 128], f32)
        nc.vector.scalar_tensor_tensor(s80[:], qc[:, 0:128], 80.0,
                                       qc[:, 128:256], op0=A.mult, op1=A.add)
        ii = {}
        for yc in (0, 1):
            tf = scp.tile([117, 128], f32, tag=f"if{yc}", name=f"if{yc}")
            nc.vector.tensor_scalar(out=tf[:], in0=s80[:],
                                    scalar1=float(80 * yc - 5103), scalar2=None,
                                    op0=A.add)
            ti = scp.tile([117, 128], i16, tag=f"ii{yc}", name=f"iit{yc}")
            nc.vector.tensor_copy(ti[:], tf[:])
            ii[yc] = ti

        coef, idxw = io["coef"], io["idxw"]
        for yc in (0, 1):
            for xc in (0, 1):
                nc.sync.dma_start(
                    out=coef[yc][:, xc * PIXP:(xc + 1) * PIXP].rearrange(
                        "p (g c) -> p g c", c=128),
                    in_=cc[(yc, xc)][:])
            # wrap to DRAM in 16-partition wrap order, then 8 replica loads
            nc.sync.dma_start(
                out=rap(dram["widx"][:], [[104, 9], [8, 13], [1, 8], [936, 16]],
                        extra_off=yc * 16 * 936),
                in_=ii[yc][:].rearrange("p (t q) -> p t q", q=16))
        for yc in (0, 1):
            for r in range(8):
                nc.sync.dma_start(
                    out=idxw[yc][r * 16:(r + 1) * 16, :],
                    in_=rap(dram["widx"][:], [[936, 16], [1, 936]],
                            extra_off=yc * 16 * 936))
        if DEBUG:
            nc.sync.dma_start(out=io["dbg_c0"][:], in_=coef[0][:])
            nc.sync.dma_start(out=io["dbg_i0"][:], in_=ii[0][:])

    # ---------- main gather / scale / matmul ----------
    src_ap_base, npos, elem, estep = io["gsrc"]
    gather_in = rap(src_ap_base, [[estep, npos], [1, elem]])
    coef, idxw = io["coef"], io["idxw"]
    with tc.tile_pool(name=f"{name}_gt", bufs=1) as gtp, \
         tc.tile_pool(name=f"{name}_cs", bufs=2) as csp, \
         tc.tile_pool(name=f"{name}_cps", bufs=2, space="PSUM") as cps, \
         tc.tile_pool(name=f"{name}_yps", bufs=2, space="PSUM") as yps:
        for pss, (poff, pn) in enumerate(PASSES):
            gts = {}
            for k in range(KK):
                for yc in (0, 1):
                    woff = PASS_WOFF[poff]
                    wn = pn // 16
                    gt = gtp.tile([128, 2 * CH, pn], bf16, tag=f"g{k}_{yc}", name=f"g{k}_{yc}")
                    nc.gpsimd.dma_gather(
                        out_ap=gt[:], in_ap=gather_in,
                        idxs_ap=idxw[yc][:, k * 104 + woff:k * 104 + woff + wn],
                        num_idxs=pn, num_idxs_reg=pn,
                        elem_size=elem, elem_step=estep, transpose=True)
                    gts[(k, yc)] = gt
                    csb = csp.tile([128, 2, pn], bf16, tag="csb", name="csb")
                    for xc in (0, 1):
                        pc = cps.tile([128, 1024], f32, tag="pc", name="pc")
                        for (coff, cn) in PASS_NCHUNKS[pn]:
                            nc.tensor.matmul(
                                pc[:, coff:coff + cn],
                                cst["sel"][:, k * 128:(k + 1) * 128],
                                coef[yc][0:9, xc * PIXP + poff + coff:
                                         xc * PIXP + poff + coff + cn],
                                start=True, stop=True)
                        nc.scalar.activation(csb[:, xc, :], pc[:, 0:pn], AF.Copy)
                    gv = gt[:].rearrange("p (a b) c -> p a b c", b=CH)
                    nc.vector.tensor_tensor(
                        gv, gv,
                        csb[:].unsqueeze(2).broadcast_to([128, 2, CH, pn]),
                        A.mult)
            for mb in range(MB):
                for (coff, cn) in PASS_NCHUNKS[pn]:
                    py = yps.tile([128, 512], f32, tag="py", name="py")
                    nmm = KK * 2 * 2 * CH
                    i = 0
                    for k in range(KK):
                        for yc in (0, 1):
                            for xc in (0, 1):
                                for ch in range(CH):
                                    t = k * CH + ch if MB == 1 else k * MB + mb
                                    lhsT = cst[f"ww{li}"][:,
                                                          t * 128:(t + 1) * 128]
                                    rhs = gts[(k, yc)][:, xc * CH + ch,
                                                       coff:coff + cn]
                                    i += 1
                                    nc.tensor.matmul(py[:, :cn], lhsT, rhs,
                                                     start=(i == 1),
                                                     stop=(i == nmm))
                    io["evict"](mb, poff + coff, cn, py)


def _build_program():
    if "nc" in _CACHE:
        return _CACHE["nc"], _CACHE["innames"]
    nc = bacc.Bacc("TRN2", target_bir_lowering=False, num_devices=8)

    ins = {}

    def di(name, shape, dt):
        ins[name] = nc.dram_tensor(name, shape, dt, kind="ExternalInput")
        return ins[name]

    di("xsq1", [128, 2 * SQ], bf16)
    di("xcl1", [NPAD, 256], bf16)
    di("xres", [128, 2 * PIX], f32)
    di("base_pk", [117, 256], f32)
    di("qsel", [27, 4], f32)
    di("selin", [9, 9 * 128], bf16)
    di("hsel", [128, 16], bf16)
    for li in ("1", "2"):
        nt = 18 if li == "1" else 9
        di(f"offw{li}", [128, nt * 27], bf16)
        di(f"ww{li}", [128, 18 * 128], bf16)
        di(f"ob{li}", [27, 1], f32)
        di(f"gapw{li}", [27, 27], f32)
        di(f"hww{li}", [27, 81], f32)
        di(f"poolw{li}", [27, 27], f32)
        for nm in ("gaps", "gapb", "hws", "hwb", "pools", "poolb"):
            di(f"{nm}{li}", [27, 1], f32)
    di("bns1", [128, 1], f32)
    di("bnb1", [128, 1], f32)
    di("bns2", [128, 2], f32)
    di("bnb2", [128, 2], f32)

    out_t = nc.dram_tensor("out", [2, 128, PIX], f32, kind="ExternalOutput")
    dbg = {}
    if DEBUG:
        for li in ("1", "2"):
            dbg[f"dbg_om{li}"] = nc.dram_tensor(f"dbg_om{li}", [27, PIXP], f32,
                                                kind="ExternalOutput")
            dbg[f"dbg_c0{li}"] = nc.dram_tensor(f"dbg_c0{li}", [9, 2 * PIXP],
                                                bf16, kind="ExternalOutput")
            dbg[f"dbg_i0{li}"] = nc.dram_tensor(f"dbg_i0{li}", [117, 128], i16,
                                                kind="ExternalOutput")
        dbg["dbg_x2"] = nc.dram_tensor("dbg_x2", [128, PIXP], bf16,
                                       kind="ExternalOutput")

    with TileContext(nc) as tc:
        from contextlib import ExitStack
        with ExitStack() as ctx:
            cstp = ctx.enter_context(tc.tile_pool(name="const", bufs=1))
            dramp = ctx.enter_context(
                tc.tile_pool(name="dram", bufs=1, space="DRAM"))
            outer = ctx.enter_context(tc.tile_pool(name="outer", bufs=1))

            cst = {}

            def ld(name, shape, dt, src=None):
                t = cstp.tile(shape, dt, tag=name, name=name)
                nc.sync.dma_start(out=t[:], in_=ins[src or name][:])
                cst[name] = t
                return t

            ld("base", [117, 256], f32, "base_pk")
            ld("qsel", [27, 4], f32)
            ld("sel", [9, 9 * 128], bf16, "selin")
            ld("hsel", [128, 16], bf16)
            for li in ("1", "2"):
                nt = 18 if li == "1" else 9
                ld(f"offw{li}", [128, nt * 27], bf16)
                ld(f"ww{li}", [128, 18 * 128], bf16)
                ld(f"ob{li}", [27, 1], f32)
                ld(f"gapw{li}", [27, 27], f32)
                ld(f"hww{li}", [27, 81], f32)
                ld(f"poolw{li}", [27, 27], f32)
                for nm in ("gaps", "gapb", "hws", "hwb", "pools", "poolb"):
                    ld(f"{nm}{li}", [27, 1], f32)
            bns1 = ld("bns1", [128, 1], f32)
            bnb1 = ld("bnb1", [128, 1], f32)
            bns2 = ld("bns2", [128, 2], f32)
            bnb2 = ld("bnb2", [128, 2], f32)

            dram = {}
            for li in ("1", "2"):
                dram[f"ag{li}_in"] = dramp.tile([27, 100], f32, tag=f"ag{li}i", name=f"ag{li}i")
                dram[f"ag{li}_out"] = dramp.tile([4, 27, 100], f32, tag=f"ag{li}o", name=f"ag{li}o")
            dram["x2in"] = dramp.tile([PIXP + 160, 128], bf16, tag="x2in", name="x2in")
            dram["x2out"] = dramp.tile([4 * (PIXP + 160), 128], bf16, tag="x2out", name="x2out")
            dram["x2pad"] = dramp.tile([NPAD, 128], bf16, tag="x2pad", name="x2pad")
            dram["widx"] = dramp.tile([2 * 16 * 936], i16, tag="widx", name="widx")

            coef = {yc: outer.tile([9, 2 * PIXP], bf16, tag=f"cf{yc}", name=f"cf{yc}")
                    for yc in (0, 1)}
            idxw = {yc: outer.tile([128, 936], i16, tag=f"iw{yc}", name=f"iw{yc}")
                    for yc in (0, 1)}
            x2sb = outer.tile([128, PIXP], bf16)

            # ---------------- layer 1 ----------------
            def evict1(mb, goff, n, py):
                nc.scalar.activation(x2sb[:, goff:goff + n], py[:, :n], AF.Silu,
                                     bias=bnb1[:], scale=bns1[:])

            io1 = {"xsq": None, "xsq_src": ins["xsq1"], "coef": coef,
                   "idxw": idxw,
                   "gsrc": (ins["xcl1"][:], NPAD - 1, 512, 256),
                   "evict": evict1}
            if DEBUG:
                io1.update({"dbg_om": dbg["dbg_om1"], "dbg_c0": dbg["dbg_c01"],
                            "dbg_i0": dbg["dbg_i01"]})
            _emit_layer(nc, tc, 1, cst, dram,
                        {"CH": 2, "MB": 1, "Cin": 256, "Cout": 128}, io1)
            if DEBUG:
                nc.sync.dma_start(out=dbg["dbg_x2"][:], in_=x2sb[:])

            # ---------------- x2 distribution ----------------
            with tc.tile_pool(name="x2t", bufs=1) as x2t:
                tr = x2t.tile([128, 13 * 128], bf16)
                for j in range(13):
                    nc.sync.dma_start_transpose(
                        tr[:, j * 128:(j + 1) * 128],
                        x2sb[:, j * 128:(j + 1) * 128])
                nc.sync.dma_start(
                    out=rap(dram["x2in"][:], [[128, 128], [16384, 13], [1, 128]]),
                    in_=tr[:].rearrange("p (b c) -> p b c", c=128))
                # halos channels-first at tail: rows [PIXP, PIXP+160)
                nc.sync.dma_start(
                    out=rap(dram["x2in"][:], [[160, 128], [1, 80]],
                            extra_off=PIXP * 128),
                    in_=x2sb[:, 0:80])
                nc.sync.dma_start(
                    out=rap(dram["x2in"][:], [[160, 128], [1, 80]],
                            extra_off=PIXP * 128 + 80),
                    in_=x2sb[:, 1520:1600])
            nc.gpsimd.collective_compute(
                "AllGather", A.bypass, replica_groups=[[0, 1, 2, 3], [4, 5, 6, 7]],
                ins=[dram["x2in"].opt()], outs=[dram["x2out"].opt()])

            with tc.tile_pool(name="x2r", bufs=1) as x2r:
                # zero pads of x2pad
                zt = x2r.tile([1, 128], bf16)
                nc.vector.memset(zt[:], 0.0)
                nc.sync.dma_start(
                    out=rap(dram["x2pad"][:], [[128, 81], [1, 128]]),
                    in_=zt[:].unsqueeze(1).broadcast_to([1, 81, 128]))
                nc.sync.dma_start(
                    out=rap(dram["x2pad"][:], [[128, 81], [1, 128]],
                            extra_off=(W + 1 + H * W) * 128),
                    in_=zt[:].unsqueeze(1).broadcast_to([1, 81, 128]))
                nc.sync.dma_start(
                    out=rap(dram["x2pad"][:], [[1600 * 128, 4], [1, 1600 * 128]],
                            extra_off=(W + 1) * 128),
                    in_=rap(dram["x2out"][:], [[BLK, 4], [1, 1600 * 128]]))
                # halo rows -> select my neighbours' rows
                hsb = x2r.tile([128, 640], bf16)
                nc.sync.dma_start(
                    out=hsb[:],
                    in_=rap(dram["x2out"][:], [[160, 128], [BLK, 4], [1, 160]],
                            extra_off=PIXP * 128))
                hv = hsb[:].rearrange("p (r c) -> p r c", c=80)   # [128, 8, 80]
                htmp = x2r.tile([128, 640], bf16)
                nc.vector.tensor_tensor(
                    htmp[:].rearrange("p (r c) -> p r c", c=80), hv,
                    cst["hsel"][:, 0:8].unsqueeze(2).broadcast_to([128, 8, 80]),
                    A.mult)
                halo_t = x2r.tile([128, 80], bf16)
                with nc.allow_low_precision(reason="one-hot halo select"):
                    nc.vector.tensor_reduce(
                        halo_t[:], htmp[:].rearrange("p (r c) -> p c r", c=80),
                        axis=AX.X, op=A.add)
                nc.vector.tensor_tensor(
                    htmp[:].rearrange("p (r c) -> p r c", c=80), hv,
                    cst["hsel"][:, 8:16].unsqueeze(2).broadcast_to([128, 8, 80]),
                    A.mult)
                halo_b = x2r.tile([128, 80], bf16)
                with nc.allow_low_precision(reason="one-hot halo select"):
                    nc.vector.tensor_reduce(
                        halo_b[:], htmp[:].rearrange("p (r c) -> p c r", c=80),
                        axis=AX.X, op=A.add)
                # assemble xsq2
                xsq2 = outer.tile([128, SQ], bf16)
                nc.vector.memset(xsq2[:], 0.0)
                nc.vector.tensor_copy(
                    xsq2[:, 83:83 + 20 * 82].rearrange("p (r c) -> p r c", c=82)
                    [:, :, 0:80],
                    x2sb[:, 0:PIX].rearrange("p (r c) -> p r c", c=80))
                nc.vector.tensor_copy(xsq2[:, 1:81], halo_t[:])
                nc.vector.tensor_copy(xsq2[:, 21 * 82 + 1:21 * 82 + 81],
                                      halo_b[:])

            # ---------------- layer 2 ----------------
            evp = ctx.enter_context(tc.tile_pool(name="evp", bufs=3))

            def evict2(mb, goff, n, py):
                nr = min(goff + n, PIX) - goff
                if nr <= 0:
                    return
                ysb = evp.tile([128, 512], f32, tag="ysb", name="ysb")
                nc.scalar.activation(ysb[:, :n], py[:, :n], AF.Silu,
                                     bias=bnb2[:, mb:mb + 1],
                                     scale=bns2[:, mb:mb + 1])
                xrt = evp.tile([128, 512], f32, tag="xrt", name="xrt")
                nc.sync.dma_start(
                    out=xrt[:, :nr],
                    in_=ins["xres"][:, mb * PIX + goff:mb * PIX + goff + nr])
                osb = evp.tile([128, 512], f32, tag="osb", name="osb")
                nc.vector.tensor_tensor(osb[:, :nr], ysb[:, :nr], xrt[:, :nr],
                                        A.add)
                nc.sync.dma_start(
                    out=out_t[mb][:, goff:goff + nr], in_=osb[:, :nr])

            io2 = {"xsq": xsq2[:], "coef": coef, "idxw": idxw,
                   "gsrc": (dram["x2pad"][:], NPAD - 1, 256, 128),
                   "evict": evict2}
            if DEBUG:
                io2.update({"dbg_om": dbg["dbg_om2"], "dbg_c0": dbg["dbg_c02"],
                            "dbg_i0": dbg["dbg_i02"]})
            _emit_layer(nc, tc, 2, cst, dram,
                        {"CH": 1, "MB": 2, "Cin": 128, "Cout": 256}, io2)


    nc.compile()
    _CACHE["nc"] = nc
    _CACHE["innames"] = list(ins.keys())
    return nc, _CACHE["innames"]


# ================================================================== host run ==

def _prep_inputs(x, cv1_params, cv2_params):
    x = np.asarray(x, np.float32)
    B = x.shape[0]
    lay1 = _prep_layer(cv1_params)
    lay2 = _prep_layer(cv2_params)
    sel = _make_sel()
    shared = {"selin": sel}
    for li, lay in (("1", lay1), ("2", lay2)):
        shared[f"offw{li}"] = lay["offw"]
        shared[f"ww{li}"] = lay["ww"]
        shared[f"ob{li}"] = lay["ob"]
        shared[f"gapw{li}"] = lay["gapw"]
        shared[f"hww{li}"] = lay["hww"]
        shared[f"poolw{li}"] = lay["poolw"]
        for nm in ("gaps", "gapb", "hws", "hwb", "pools", "poolb"):
            shared[f"{nm}{li}"] = lay[nm]
    shared["bns1"] = lay1["bns"]
    shared["bnb1"] = lay1["bnb"]
    shared["bns2"] = np.ascontiguousarray(lay2["bns"].reshape(2, 128).T)
    shared["bnb2"] = np.ascontiguousarray(lay2["bnb"].reshape(2, 128).T)

    xcl = [np.ascontiguousarray(_make_xcl(x[b])) for b in range(B)]
    in_maps = []
    for core in range(8):
        b, q = core // 4, core % 4
        r0 = q * ROWS
        m = dict(shared)
        m["xsq1"] = _make_xsq(x[b], r0)
        m["xcl1"] = xcl[b]
        m["xres"] = np.ascontiguousarray(np.transpose(
            x[b].reshape(2, 128, H, W)[:, :, r0:r0 + ROWS, :].reshape(
                2, 128, PIX), (1, 0, 2)).reshape(128, 2 * PIX))
        m["base_pk"] = _make_base_pk(r0)
        qs = np.zeros((27, 4), np.float32)
        qs[:, q] = 1.0
        m["qsel"] = qs
        hs = np.zeros((128, 16), np.float32)
        if q > 0:
            hs[:, (q - 1) * 2 + 1] = 1.0       # top halo: prev rank, last row
        if q < 3:
            hs[:, 8 + (q + 1) * 2] = 1.0       # bottom halo: next rank, 1st row
        m["hsel"] = hs.astype(bfloat16)
        in_maps.append(m)
    return in_maps


def kernel(**inputs):
    nc, _ = _build_program()
    in_maps = _prep_inputs(**inputs)
    res = run_bass_kernel_spmd(nc, in_maps, list(range(8)))
    x = np.asarray(inputs["x"], np.float32)
    out = np.empty_like(x)
    for core in range(8):
        b, q = core // 4, core % 4
        r0 = q * ROWS
        o = res.results[core]["out"].reshape(2, 128, ROWS, W)
        out[b, :, r0:r0 + ROWS, :] = o.reshape(256, ROWS, W)
    return out


def run_with_results(inputs, trace=False):
    """test harness helper: returns (output, BassKernelResults)."""
    nc, _ = _build_program()
    in_maps = _prep_inputs(**inputs)
    res = run_bass_kernel_spmd(nc, in_maps, list(range(8)), trace=trace)
    x = np.asarray(inputs["x"], np.float32)
    out = np.empty_like(x)
    for core in range(8):
        b, q = core // 4, core % 4
        r0 = q * ROWS
        o = res.results[core]["out"].reshape(2, 128, ROWS, W)
        out[b, :, r0:r0 + ROWS, :] = o.reshape(256, ROWS, W)
    return out, res
